# revision 1
# baseline (speedup 1.0000x reference)
"""Trainium2 Bass kernel for nn_CharAttention (causal single-head attention, T=4096, D=1024).

Strategy (8 NeuronCores, SPMD):
  - Queries sharded across cores with a balanced causal interleave: core c owns
    global 128-row q-blocks {c, 15-c, 16+c, 31-c} ("slots" 0..3), so every core
    does the same causal work (structurally identical static program).
  - k computed on a contiguous 512-key shard per core, v on an INTERLEAVED key
    shard (core c owns 128-key blocks {c, 8+c, 16+c, 24+c}); one AllGather each
    (each collective carries a ~15us fixed cost on the serial CC engine, so
    fewer/bigger is better; k must fully land before any score tile anyway).
  - Projections run d-outer with 8 PSUM banks accumulating in parallel, so all
    8 output stores complete right after the last input chunk lands and the
    AllGather triggers as early as possible.
  - Input/static loads are split into 32-partition chunks: a DMA runs on a
    single ring (~14 GB/s), so chunking across rings cuts per-tile latency 4x.
  - Slot s is padded to (s+1)*1024 key-columns; the data-dependent causal
    boundary is applied with an iota>pos additive -1e9 mask on the last quarter
    of each slot (the diagonal always lands there for every core).
  - Softmax without a running max: m_hat = rowmax(first 1024 cols) + 50.
    exp(s - m_hat) stays within bf16/f32 range, so quarter contributions
    accumulate with plain adds and one final 1/l normalization.
  - dtypes: q/k/scores chain in fp16 (PE full rate, fine mantissa -- scores
    have std ~32 and softmax is argmax-sensitive); p/v/attn/proj in bf16.
"""

import numpy as np
import ml_dtypes

T = 4096
D = 1024
N_CORES = 8
NBLK = T // 128  # 32 global q-blocks
DELTA = 50.0
NEG_BIG = -1e9

# q slot assignment: core c -> global blocks [c, 15-c, 16+c, 31-c]
def core_blocks(c):
    return [c, 15 - c, 16 + c, 31 - c]

# kv interleaved shard: core c -> key blocks [c, 8+c, 16+c, 24+c]
def kv_blocks(c):
    return [c, 8 + c, 16 + c, 24 + c]

PADQ = [1, 2, 3, 4]  # quarters (1024 cols) computed per slot

_COMPILED = None
LAST_EXEC_NS = None
LAST_RES = None


def _build():
    import concourse.bass as bass
    import concourse.mybir as mybir
    from concourse import bacc
    from concourse.tile import TileContext
    from concourse.masks import make_identity

    f16, bf16, f32 = mybir.dt.float16, mybir.dt.bfloat16, mybir.dt.float32
    AT = mybir.ActivationFunctionType
    OP = mybir.AluOpType
    AX = mybir.AxisListType

    nc = bacc.Bacc("TRN2", target_bir_lowering=False, debug=False, num_devices=N_CORES)

    # --- I/O ---
    xqT_d = nc.dram_tensor("xqT", [D, 512], f16, kind="ExternalInput")
    xkT_d = nc.dram_tensor("xkT", [D, 512], f16, kind="ExternalInput")
    xvT_d = nc.dram_tensor("xvT", [D, 512], f16, kind="ExternalInput")
    xqres_d = nc.dram_tensor("xqres", [512, D], f16, kind="ExternalInput")
    pos_d = nc.dram_tensor("pos", [128, 4], f32, kind="ExternalInput")
    wqT_d = nc.dram_tensor("wqT", [D, D], f16, kind="ExternalInput")
    wkT_d = nc.dram_tensor("wkT", [D, D], f16, kind="ExternalInput")
    wvT_d = nc.dram_tensor("wvT", [D, D], f16, kind="ExternalInput")
    wpT_d = nc.dram_tensor("wpT", [D, D], bf16, kind="ExternalInput")
    out_d = nc.dram_tensor("out", [512, D], bf16, kind="ExternalOutput")
    attn_d = nc.dram_tensor("attn", [512, D], bf16, kind="ExternalOutput")

    ktloc_d = nc.dram_tensor("ktloc", [D, 512], f16)
    vloc_d = nc.dram_tensor("vloc", [512, D], bf16)
    ktag_d = nc.dram_tensor("ktag", [N_CORES, D, 512], f16, addr_space="Shared")
    vag_d = nc.dram_tensor("vag", [N_CORES, 512, D], bf16, addr_space="Shared")

    groups = [list(range(N_CORES))]

    def load_split(tile, dram, row0, ncols, nchunks=4):
        """Split a [128, ncols] tile load into partition chunks across rings."""
        step = 128 // nchunks
        for a in range(nchunks):
            nc.sync.dma_start(
                out=tile[step * a:step * (a + 1), 0:ncols],
                in_=dram[row0 + step * a:row0 + step * (a + 1), :],
            )

    with TileContext(nc) as tc:
        with tc.tile_pool(name="persist", bufs=1) as pp:
            # persistent tiles
            qt = [pp.tile([128, 512], f16, tag=f"qt{e}", name=f"qt{e}") for e in range(8)]
            ident = pp.tile([128, 128], bf16, tag="ident", name="ident")
            make_identity(nc, ident[:])
            iota_i = pp.tile([128, 512], mybir.dt.int32, tag="iota_i", name="iota_i")
            nc.gpsimd.iota(iota_i[:], pattern=[[1, 512]], base=0, channel_multiplier=0)
            iota_f = pp.tile([128, 512], f32, tag="iota_f", name="iota_f")
            nc.vector.tensor_copy(iota_f[:], iota_i[:])
            pos_sb = pp.tile([128, 4], f32, tag="pos_sb", name="pos_sb")
            nc.sync.dma_start(out=pos_sb[:], in_=pos_d[:])
            A_sb = [pp.tile([128, D], f32, tag=f"A{s}", name=f"A{s}") for s in range(4)]
            lpart = [pp.tile([128, 4], f32, tag=f"lp{s}", name=f"lp{s}") for s in range(4)]
            negm = [pp.tile([128, 1], f32, tag=f"nm{s}", name=f"nm{s}") for s in range(4)]
            rl_p = [pp.tile([128, 1], f32, tag=f"rl{s}", name=f"rl{s}") for s in range(4)]

            # ---------------- Phase A: projections + allgathers ----------------
            with (
                tc.tile_pool(name="pha", bufs=1) as pa,
                tc.tile_pool(name="psa", bufs=1, space="PSUM") as psa,
            ):
                xkt = [pa.tile([128, 512], f16, tag=f"xkt{d}", name=f"xkt{d}") for d in range(8)]
                wk = [pa.tile([128, D], f16, tag=f"wk{d}", name=f"wk{d}") for d in range(8)]
                for d in range(8):
                    load_split(wk[d], wkT_d, 128 * d, D)
                    load_split(xkt[d], xkT_d, 128 * d, 512)
                xvt = [pa.tile([128, 512], f16, tag=f"xvt{d}", name=f"xvt{d}") for d in range(8)]
                wv = [pa.tile([128, D], f16, tag=f"wv{d}", name=f"wv{d}") for d in range(8)]
                for d in range(8):
                    load_split(wv[d], wvT_d, 128 * d, D)
                    load_split(xvt[d], xvT_d, 128 * d, 512)
                xqt = [pa.tile([128, 512], f16, tag=f"xqt{d}", name=f"xqt{d}") for d in range(8)]
                wq = [pa.tile([128, D], f16, tag=f"wq{d}", name=f"wq{d}") for d in range(8)]
                for d in range(8):
                    load_split(wq[d], wqT_d, 128 * d, D)
                    load_split(xqt[d], xqT_d, 128 * d, 512)

                with nc.named_scope("k_proj"):
                    # kT_local[e,t] = sum_d WkT[d,e] * xkT[d,t]
                    for e in range(8):
                        ps = psa.tile([128, 512], f32, tag="kp", name="kp", bufs=3)
                        for d in range(8):
                            nc.tensor.matmul(
                                ps[:], wk[d][:, 128 * e:128 * (e + 1)], xkt[d][:],
                                start=(d == 0), stop=(d == 7),
                            )
                        kt_sb = pa.tile([128, 512], f16, tag="kt_sb", name="kt_sb", bufs=4)
                        nc.scalar.copy(kt_sb[:], ps[:])
                        nc.scalar.dma_start(
                            out=ktloc_d[128 * e:128 * (e + 1), :], in_=kt_sb[:],
                        )
                    nc.gpsimd.collective_compute(
                        "AllGather", mybir.AluOpType.bypass, replica_groups=groups,
                        ins=[ktloc_d[:]], outs=[ktag_d[:]],
                    )

                with nc.named_scope("v_proj"):
                    # v_local[t,e] = sum_d xvT[d,t] * WvT[d,e]
                    for j in range(4):
                        for h2 in range(2):
                            ps = psa.tile([128, 512], f32, tag="kp", name="kp", bufs=3)
                            for d in range(8):
                                nc.tensor.matmul(
                                    ps[:], xvt[d][:, 128 * j:128 * (j + 1)],
                                    wv[d][:, 512 * h2:512 * (h2 + 1)],
                                    start=(d == 0), stop=(d == 7),
                                )
                            v_sb = pa.tile([128, 512], bf16, tag="v_sb", name="v_sb", bufs=4)
                            nc.scalar.copy(v_sb[:], ps[:])
                            nc.scalar.dma_start(
                                out=vloc_d[128 * j:128 * (j + 1), 512 * h2:512 * (h2 + 1)],
                                in_=v_sb[:],
                            )
                    nc.gpsimd.collective_compute(
                        "AllGather", mybir.AluOpType.bypass, replica_groups=groups,
                        ins=[vloc_d[:]], outs=[vag_d[:]],
                    )

                with nc.named_scope("q_proj"):
                    for e in range(8):
                        ps = psa.tile([128, 512], f32, tag="kp", name="kp", bufs=3)
                        for d in range(8):
                            nc.tensor.matmul(
                                ps[:], wq[d][:, 128 * e:128 * (e + 1)], xqt[d][:],
                                start=(d == 0), stop=(d == 7),
                            )
                        nc.scalar.copy(qt[e][:], ps[:])

            # ---------------- Phase B: attention over quarters ----------------
            with (
                tc.tile_pool(name="phb", bufs=1) as pb,
                tc.tile_pool(name="psb", bufs=1, space="PSUM") as psp,
            ):
                wproj = [pb.tile([128, D], bf16, tag=f"wp{d}", name=f"wp{d}") for d in range(8)]
                for d in range(8):
                    load_split(wproj[d], wpT_d, 128 * d, D)
                xqr = [pb.tile([128, D], f16, tag=f"xqr{s}", bufs=1, name=f"xqr{s}") for s in range(4)]
                for s in range(4):
                    load_split(xqr[s], xqres_d, 128 * s, D)

                # ---- Pass 1: all scores + exp + P-transposes ----
                # The PE transpose of P(s) depends on the scalar Exp(s); emit it
                # one step late so the next slot's score matmuls hide the
                # exp latency instead of stalling the PE queue.
                pt_tiles = {}
                pending_tr = []

                def emit_tr():
                    s_, qtr_, psb_ = pending_tr.pop(0)
                    ps_tr = psp.tile([128, 1024], bf16, tag="ps_trrow", bufs=1, name="ps_trrow")
                    for j in range(8):
                        nc.tensor.transpose(
                            ps_tr[:, 128 * j:128 * (j + 1)],
                            psb_[:, 128 * j:128 * (j + 1)], ident[:])
                    pt_sb = pb.tile([128, 1024], bf16, tag="pt_sb", bufs=10, name="pt_sb")
                    nc.scalar.copy(pt_sb[:], ps_tr[:])
                    pt_tiles[(s_, qtr_)] = pt_sb

                for qtr in range(4):
                    with nc.named_scope(f"sc{qtr}"):
                        kq = [pb.tile([128, 1024], f16, tag=f"kq{e}", bufs=3, name=f"kq{e}") for e in range(8)]
                        for e in range(8):
                            for hh in range(2):
                                cc = 2 * qtr + hh
                                for a in range(2):
                                    nc.sync.dma_start(
                                        out=kq[e][64 * a:64 * (a + 1), 512 * hh:512 * (hh + 1)],
                                        in_=ktag_d[cc, 128 * e + 64 * a:128 * e + 64 * (a + 1), :],
                                    )
                        for s in range(4):
                            if qtr >= PADQ[s]:
                                continue
                            last_q = (qtr == PADQ[s] - 1)
                            ssb = pb.tile([128, 1024], f32, tag="ssb", bufs=2, name="ssb")
                            for pn in range(2):
                                ps = psp.tile([128, 512], f32, tag="pp", name="pp", bufs=3)
                                for e in range(8):
                                    nc.tensor.matmul(
                                        ps[:], qt[e][:, 128 * s:128 * (s + 1)],
                                        kq[e][:, 512 * pn:512 * (pn + 1)],
                                        start=(e == 0), stop=(e == 7),
                                    )
                                if last_q:
                                    shift = pb.tile([128, 1], f32, tag="shift", bufs=2, name="shift")
                                    nc.vector.tensor_scalar_add(
                                        shift[:], pos_sb[:, s:s + 1],
                                        float(-(qtr * 1024 + pn * 512)),
                                    )
                                    madd = pb.tile([128, 512], f32, tag="madd", bufs=1, name="madd")
                                    nc.vector.tensor_scalar(
                                        out=madd[:], in0=iota_f[:], scalar1=shift[:, 0:1],
                                        scalar2=NEG_BIG, op0=OP.is_gt, op1=OP.mult,
                                    )
                                    nc.vector.tensor_tensor(
                                        out=ssb[:, 512 * pn:512 * (pn + 1)], in0=ps[:],
                                        in1=madd[:], op=OP.add,
                                    )
                                else:
                                    nc.scalar.copy(ssb[:, 512 * pn:512 * (pn + 1)], ps[:])
                            if qtr == 0:
                                nc.vector.reduce_max(
                                    negm[s][:], ssb[:], axis=AX.X, negate=True)
                                nc.vector.tensor_scalar_add(negm[s][:], negm[s][:], -DELTA)
                            psb = pb.tile([128, 1024], bf16, tag="psb", bufs=3, name="psb")
                            nc.scalar.activation(
                                psb[:], ssb[:], AT.Exp, bias=negm[s][:, 0:1], scale=1.0,
                                accum_out=lpart[s][:, qtr:qtr + 1],
                            )
                            if last_q:
                                # 1/l ready well before fin needs it
                                lsum = pb.tile([128, 1], f32, tag="lsum", bufs=2, name="lsum")
                                if PADQ[s] > 1:
                                    nc.vector.reduce_sum(
                                        lsum[:], lpart[s][:, 0:PADQ[s]], axis=AX.X)
                                else:
                                    nc.vector.tensor_copy(lsum[:], lpart[s][:, 0:1])
                                nc.vector.reciprocal(rl_p[s][:], lsum[:])
                            pending_tr.append((s, qtr, psb))
                            if len(pending_tr) > 1:
                                emit_tr()
                while pending_tr:
                    emit_tr()

                # ---- Pass 2: att @ v per quarter, then per-slot finalize ----
                # fin(s) chains PE work behind vector/scalar latency; defer each
                # fin until after the NEXT quarter's av matmuls are issued so
                # those matmuls hide the latency.
                def do_fin(s):
                    with nc.named_scope(f"fin{s}"):
                        attn_b = pb.tile([128, D], bf16, tag="attn_b", bufs=2, name="attn_b")
                        nc.scalar.activation(
                            attn_b[:], A_sb[s][:], AT.Copy, bias=0.0,
                            scale=rl_p[s][:, 0:1])
                        for a in range(2):
                            nc.sync.dma_start(
                                out=attn_d[128 * s + 64 * a:128 * s + 64 * (a + 1), :],
                                in_=attn_b[64 * a:64 * (a + 1), :])
                        ps_t2 = psp.tile([128, 1024], bf16, tag="ps_trrow", bufs=1, name="ps_trrow")
                        for ec in range(8):
                            nc.tensor.transpose(
                                ps_t2[:, 128 * ec:128 * (ec + 1)],
                                attn_b[:, 128 * ec:128 * (ec + 1)],
                                ident[:])
                        at_row = pb.tile([128, 1024], bf16, tag="at_sb", bufs=2, name="at_sb")
                        nc.scalar.copy(at_row[:], ps_t2[:])
                        out_sb = pb.tile([128, D], bf16, tag="out_sb", bufs=2, name="out_sb")
                        for h in range(2):
                            ps_o = psp.tile([128, 512], f32, tag="pp", name="pp", bufs=3)
                            for ec in range(8):
                                nc.tensor.matmul(
                                    ps_o[:], at_row[:, 128 * ec:128 * (ec + 1)],
                                    wproj[ec][:, 512 * h:512 * (h + 1)],
                                    start=(ec == 0), stop=(ec == 7),
                                )
                            nc.vector.tensor_tensor(
                                out=out_sb[:, 512 * h:512 * (h + 1)], in0=ps_o[:],
                                in1=xqr[s][:, 512 * h:512 * (h + 1)], op=OP.add)
                            for a in range(2):
                                nc.sync.dma_start(
                                    out=out_d[128 * s + 64 * a:128 * s + 64 * (a + 1),
                                              512 * h:512 * (h + 1)],
                                    in_=out_sb[64 * a:64 * (a + 1), 512 * h:512 * (h + 1)])

                pending_fin = []
                for qtr in range(4):
                    with nc.named_scope(f"av{qtr}"):
                        vq = [pb.tile([128, 1024], bf16, tag=f"vq{j}", bufs=2, name=f"vq{j}") for j in range(8)]
                        for j in range(8):
                            for a in range(4):
                                nc.sync.dma_start(
                                    out=vq[j][32 * a:32 * (a + 1), :],
                                    in_=vag_d[j, 128 * qtr + 32 * a:128 * qtr + 32 * (a + 1), :],
                                )
                        for s in range(4):
                            if qtr >= PADQ[s]:
                                continue
                            pt_sb = pt_tiles[(s, qtr)]
                            ps_av = psp.tile([128, 1024], f32, tag="ps_av", name="ps_av", bufs=2)
                            for h in range(2):
                                for j in range(8):
                                    nc.tensor.matmul(
                                        ps_av[:, 512 * h:512 * (h + 1)],
                                        pt_sb[:, 128 * j:128 * (j + 1)],
                                        vq[j][:, 512 * h:512 * (h + 1)],
                                        start=(j == 0), stop=(j == 7),
                                    )
                            if qtr == 0:
                                nc.vector.tensor_copy(A_sb[s][:], ps_av[:])
                            else:
                                nc.vector.tensor_tensor(
                                    out=A_sb[s][:], in0=A_sb[s][:], in1=ps_av[:], op=OP.add)
                        while pending_fin:
                            do_fin(pending_fin.pop(0))
                        pending_fin.append(qtr)  # slot s==qtr finishes at qtr
                while pending_fin:
                    do_fin(pending_fin.pop(0))

    nc.compile()
    return nc


def _get_compiled():
    global _COMPILED
    if _COMPILED is None:
        _COMPILED = _build()
    return _COMPILED


def kernel(x, attention_mask, Wq, Wkv, Wproj, _trace=False):
    global LAST_EXEC_NS, LAST_RES
    from concourse.bass_utils import run_bass_kernel_spmd

    x = np.asarray(x)
    attention_mask = np.asarray(attention_mask)
    Wq, Wkv, Wproj = np.asarray(Wq), np.asarray(Wkv), np.asarray(Wproj)
    assert x.shape == (T, D) and attention_mask.shape == (T,)
    assert np.array_equal(attention_mask, np.arange(T, dtype=attention_mask.dtype)), \
        "kernel assumes attention_mask == arange(T)"

    x16 = x.astype(np.float16)
    wqT = np.ascontiguousarray(Wq.T).astype(np.float16)
    wkT = np.ascontiguousarray(Wkv[:D].T).astype(np.float16)
    wvT = np.ascontiguousarray(Wkv[D:].T).astype(np.float16)
    wpT = np.ascontiguousarray(Wproj.T).astype(ml_dtypes.bfloat16)

    in_maps = []
    core_rows = []
    for c in range(N_CORES):
        blocks = core_blocks(c)
        rows = np.concatenate([np.arange(128 * b, 128 * (b + 1)) for b in blocks])
        core_rows.append(rows)
        xqT = np.ascontiguousarray(x16[rows].T)    # [D, 512] f16
        xkT = np.ascontiguousarray(x16[512 * c:512 * (c + 1)].T)
        vrows = np.concatenate([np.arange(128 * b, 128 * (b + 1)) for b in kv_blocks(c)])
        xvT = np.ascontiguousarray(x16[vrows].T)
        pos = np.empty((128, 4), np.float32)
        for s, b in enumerate(blocks):
            pos[:, s] = 128 * b + np.arange(128)
        in_maps.append({
            "xqT": xqT, "xkT": xkT, "xvT": xvT,
            "xqres": np.ascontiguousarray(x16[rows]),
            "pos": pos,
            "wqT": wqT, "wkT": wkT, "wvT": wvT, "wpT": wpT,
        })

    nc = _get_compiled()
    res = run_bass_kernel_spmd(nc, in_maps, list(range(N_CORES)), trace=_trace)
    LAST_EXEC_NS = res.exec_time_ns
    LAST_RES = res

    out_full = np.empty((T, D), np.float32)
    x_new = x.astype(np.float32).copy()
    for c in range(N_CORES):
        r = res.results[c]
        out_full[core_rows[c]] = r["out"].astype(np.float32)
        x_new[core_rows[c]] += r["attn"].astype(np.float32)
    return out_full, x_new



# revision 5
# speedup vs baseline: 1.6537x; 1.6537x over previous
"""Trainium2 Bass kernel for nn_CharAttention (causal single-head attention, T=4096, D=1024).

Strategy (8 NeuronCores, SPMD), v2:
  - Queries sharded across cores with a balanced causal interleave: core c owns
    global 128-row q-blocks {c, 15-c, 16+c, 31-c} ("slots" 0..3), so every core
    does the same causal work (structurally identical static program).
  - k computed on a contiguous 512-key shard per core, v on an INTERLEAVED key
    shard (core c owns 128-key blocks {c, 8+c, 16+c, 24+c}); one AllGather each.
  - v2: DMA count collapsed from ~520 to ~30. The HWDGE sequencer spends
    ~630ns issuing each dma_start, so the baseline's 4-way chunked loads
    saturated the Sync queue for ~330us and starved the PE. Every tensor now
    moves in ONE dma_start with a strided access pattern (rearrange), e.g.
    each weight is a single [128, 8192] tile load, each score-quarter's keys
    a single [128, 8192] gather from the AllGather output.
  - Phase A is ordered to trigger AG(k) as early as possible: load xk+wk ->
    k_proj -> single 1MB store -> AllGather, with v/q work overlapping the
    collective. Quarter-sized kq/vq tiles (bufs=2) double-buffer behind the
    score/AV passes; PSUM->SBUF copies moved to the idle Vector engine; exp
    reads PSUM directly (no f32 staging copy except for masked quarters).
  - Slot s is padded to (s+1)*1024 key-columns; the data-dependent causal
    boundary is applied with an iota>pos additive -1e9 mask on the last quarter
    of each slot (the diagonal always lands there for every core).
  - Softmax without a running max: m_hat = rowmax(first 1024 cols) + 50.
    exp(s - m_hat) stays within range, so quarter contributions accumulate
    with plain adds and one final 1/l normalization.
  - dtypes: q/k/scores chain in fp16 (PE full rate, fine mantissa); p/v/attn/
    proj in bf16.
"""

import numpy as np
import ml_dtypes

T = 4096
D = 1024
N_CORES = 8
NBLK = T // 128  # 32 global q-blocks
DELTA = 50.0
NEG_BIG = -1e9

# q slot assignment: core c -> global blocks [c, 15-c, 16+c, 31-c]
def core_blocks(c):
    return [c, 15 - c, 16 + c, 31 - c]

# kv interleaved shard: core c -> key blocks [c, 8+c, 16+c, 24+c]
def kv_blocks(c):
    return [c, 8 + c, 16 + c, 24 + c]

PADQ = [1, 2, 3, 4]  # quarters (1024 cols) computed per slot

_COMPILED = None
LAST_EXEC_NS = None
LAST_RES = None


def _build():
    import concourse.bass as bass
    import concourse.mybir as mybir
    from concourse import bacc
    from concourse.tile import TileContext
    from concourse.masks import make_identity

    f16, bf16, f32 = mybir.dt.float16, mybir.dt.bfloat16, mybir.dt.float32
    AT = mybir.ActivationFunctionType
    OP = mybir.AluOpType
    AX = mybir.AxisListType

    nc = bacc.Bacc("TRN2", target_bir_lowering=False, debug=False, num_devices=N_CORES)

    # --- I/O ---
    xqT_d = nc.dram_tensor("xqT", [D, 512], f16, kind="ExternalInput")
    xkT_d = nc.dram_tensor("xkT", [D, 512], f16, kind="ExternalInput")
    xvT_d = nc.dram_tensor("xvT", [D, 512], f16, kind="ExternalInput")
    xqres_d = nc.dram_tensor("xqres", [512, D], f16, kind="ExternalInput")
    pos_d = nc.dram_tensor("pos", [128, 4], f32, kind="ExternalInput")
    wqT_d = nc.dram_tensor("wqT", [D, D], f16, kind="ExternalInput")
    wkT_d = nc.dram_tensor("wkT", [D, D], f16, kind="ExternalInput")
    wvT_d = nc.dram_tensor("wvT", [D, D], f16, kind="ExternalInput")
    wpT_d = nc.dram_tensor("wpT", [D, D], bf16, kind="ExternalInput")
    out_d = nc.dram_tensor("out", [512, D], bf16, kind="ExternalOutput")
    attn_d = nc.dram_tensor("attn", [512, D], bf16, kind="ExternalOutput")

    ktloc_d = nc.dram_tensor("ktloc", [D, 512], f16)
    vloc_d = nc.dram_tensor("vloc", [512, D], bf16)
    ktag_d = nc.dram_tensor("ktag", [N_CORES, D, 512], f16, addr_space="Shared")
    vag_d = nc.dram_tensor("vag", [N_CORES, 512, D], bf16, addr_space="Shared")

    groups = [list(range(N_CORES))]

    with TileContext(nc) as tc:
        with tc.tile_pool(name="persist", bufs=1) as pp:
            # persistent tiles
            ident = pp.tile([128, 128], bf16, tag="ident", name="ident")
            make_identity(nc, ident[:])
            iota_f = pp.tile([128, 512], f32, tag="iota_f", name="iota_f")
            pos_sb = pp.tile([128, 4], f32, tag="pos_sb", name="pos_sb")
            qt_all = pp.tile([128, 8 * 512], f16, tag="qt", name="qt")
            wproj = pp.tile([128, 8 * 1024], bf16, tag="wp", name="wp")
            xqr = pp.tile([128, 4 * 1024], f16, tag="xqr", name="xqr")
            A_sb = [pp.tile([128, D], f32, tag=f"A{s}", name=f"A{s}") for s in range(4)]
            lpart = [pp.tile([128, 8], f32, tag=f"lp{s}", name=f"lp{s}") for s in range(4)]
            negm2 = [pp.tile([128, 2], f32, tag=f"nm2{s}", name=f"nm2{s}") for s in range(4)]
            negm = [pp.tile([128, 1], f32, tag=f"nm{s}", name=f"nm{s}") for s in range(4)]
            rl_p = [pp.tile([128, 1], f32, tag=f"rl{s}", name=f"rl{s}") for s in range(4)]

            # ---------------- Phase A: projections + allgathers ----------------
            with (
                tc.tile_pool(name="pha", bufs=1) as pa,
                tc.tile_pool(name="psa", bufs=1, space="PSUM") as psa,
            ):
                # ---- all loads: ONE dma_start per tensor, in priority order ----
                xkt = pa.tile([128, 8 * 512], f16, tag="xkt", name="xkt")
                nc.sync.dma_start(
                    out=xkt[:].rearrange("p (c t) -> p c t", c=8),
                    in_=xkT_d[:].rearrange("(c p) t -> p c t", p=128))
                wk = pa.tile([128, 8 * 1024], f16, tag="wk", name="wk")
                nc.sync.dma_start(
                    out=wk[:].rearrange("p (c e) -> p c e", c=8),
                    in_=wkT_d[:].rearrange("(c p) e -> p c e", p=128))
                xvt = pa.tile([128, 8 * 512], f16, tag="xvt", name="xvt")
                nc.sync.dma_start(
                    out=xvt[:].rearrange("p (c t) -> p c t", c=8),
                    in_=xvT_d[:].rearrange("(c p) t -> p c t", p=128))
                wv = pa.tile([128, 8 * 1024], f16, tag="wv", name="wv")
                nc.sync.dma_start(
                    out=wv[:].rearrange("p (c e) -> p c e", c=8),
                    in_=wvT_d[:].rearrange("(c p) e -> p c e", p=128))
                xqt = pa.tile([128, 8 * 512], f16, tag="xqt", name="xqt")
                nc.sync.dma_start(
                    out=xqt[:].rearrange("p (c t) -> p c t", c=8),
                    in_=xqT_d[:].rearrange("(c p) t -> p c t", p=128))
                wq = pa.tile([128, 8 * 1024], f16, tag="wq", name="wq")
                nc.sync.dma_start(
                    out=wq[:].rearrange("p (c e) -> p c e", c=8),
                    in_=wqT_d[:].rearrange("(c p) e -> p c e", p=128))
                nc.sync.dma_start(out=pos_sb[:], in_=pos_d[:])
                nc.sync.dma_start(
                    out=wproj[:].rearrange("p (c e) -> p c e", c=8),
                    in_=wpT_d[:].rearrange("(c p) e -> p c e", p=128))
                nc.sync.dma_start(
                    out=xqr[:].rearrange("p (s e) -> p s e", s=4),
                    in_=xqres_d[:].rearrange("(s p) e -> p s e", p=128))

                iota_i = pa.tile([128, 512], mybir.dt.int32, tag="iota_i", name="iota_i")
                nc.gpsimd.iota(iota_i[:], pattern=[[1, 512]], base=0, channel_multiplier=0)
                nc.vector.tensor_copy(iota_f[:], iota_i[:])

                with nc.named_scope("k_proj"):
                    # kT_local[e,t] = sum_d WkT[d,e] * xkT[d,t]
                    kt_sb = pa.tile([128, 8 * 512], f16, tag="kt_sb", name="kt_sb")
                    for e in range(8):
                        ps = psa.tile([128, 512], f32, tag="kp", name="kp", bufs=4)
                        for d in range(8):
                            nc.tensor.matmul(
                                ps[:],
                                wk[:, 1024 * d + 128 * e:1024 * d + 128 * (e + 1)],
                                xkt[:, 512 * d:512 * (d + 1)],
                                start=(d == 0), stop=(d == 7),
                            )
                        nc.vector.tensor_copy(kt_sb[:, 512 * e:512 * (e + 1)], ps[:])
                    nc.gpsimd.dma_start(
                        out=ktloc_d[:].rearrange("(c p) t -> p c t", p=128),
                        in_=kt_sb[:].rearrange("p (c t) -> p c t", c=8))
                    nc.gpsimd.collective_compute(
                        "AllGather", mybir.AluOpType.bypass, replica_groups=groups,
                        ins=[ktloc_d[:]], outs=[ktag_d[:]],
                    )

                with nc.named_scope("v_proj"):
                    # v_local[t,e] = sum_d xvT[d,t] * WvT[d,e]
                    vt_sb = pa.tile([128, 4 * 1024], bf16, tag="vt_sb", name="vt_sb")
                    for j2 in range(4):
                        for h2 in range(2):
                            ps = psa.tile([128, 512], f32, tag="kp", name="kp", bufs=4)
                            for d in range(8):
                                nc.tensor.matmul(
                                    ps[:],
                                    xvt[:, 512 * d + 128 * j2:512 * d + 128 * (j2 + 1)],
                                    wv[:, 1024 * d + 512 * h2:1024 * d + 512 * (h2 + 1)],
                                    start=(d == 0), stop=(d == 7),
                                )
                            nc.vector.tensor_copy(
                                vt_sb[:, 1024 * j2 + 512 * h2:1024 * j2 + 512 * (h2 + 1)],
                                ps[:])
                    nc.gpsimd.dma_start(
                        out=vloc_d[:].rearrange("(c p) e -> p c e", p=128),
                        in_=vt_sb[:].rearrange("p (c e) -> p c e", c=4))
                    nc.gpsimd.collective_compute(
                        "AllGather", mybir.AluOpType.bypass, replica_groups=groups,
                        ins=[vloc_d[:]], outs=[vag_d[:]],
                    )

                with nc.named_scope("q_proj"):
                    for e in range(8):
                        ps = psa.tile([128, 512], f32, tag="kp", name="kp", bufs=4)
                        for d in range(8):
                            nc.tensor.matmul(
                                ps[:],
                                wq[:, 1024 * d + 128 * e:1024 * d + 128 * (e + 1)],
                                xqt[:, 512 * d:512 * (d + 1)],
                                start=(d == 0), stop=(d == 7),
                            )
                        nc.vector.tensor_copy(qt_all[:, 512 * e:512 * (e + 1)], ps[:])

            # kq tiles live in the persist pool so their loads don't carry a
            # WAR dependency on phase A's pool space.
            # kq layout: col = 4096*pn + 512*e + t  (pn = which 512-key shard)
            kq_tiles = {}
            for qtr in range(4):
                kq_tiles[qtr] = pp.tile(
                    [128, 8 * 1024], f16, tag="kq", name="kq", bufs=2)
                for cc in range(2):
                    nc.sync.dma_start(
                        out=kq_tiles[qtr][:, 4096 * cc:4096 * (cc + 1)].rearrange(
                            "p (e t) -> p e t", e=8),
                        in_=ktag_d[2 * qtr + cc, :, :].rearrange(
                            "(e p) t -> p e t", p=128),
                    )

            # ---------------- Phase B: attention over quarters ----------------
            with (
                tc.tile_pool(name="phb", bufs=1) as pb,
                tc.tile_pool(name="psb_pool", bufs=1, space="PSUM") as psp,
            ):
                # vq loads: all 4 quarters resident (no reuse stalls); they only
                # wait on AG(v) + phase A pool space.
                vq_tiles = {}
                for qtr in range(4):
                    vq_tiles[qtr] = pb.tile(
                        [128, 8 * 1024], bf16, tag=f"vq{qtr}", name=f"vq{qtr}")
                    nc.sync.dma_start(
                        out=vq_tiles[qtr][:].rearrange("p (j e) -> p j e", j=8),
                        in_=vag_d[:, 128 * qtr:128 * (qtr + 1), :].rearrange(
                            "j p e -> p j e"),
                    )

                # ---- Pass 1: all scores + exp + P-transposes ----
                # The PE transpose of P(s) depends on the scalar Exp(s); emit it
                # one step late so the next slot's score matmuls hide the
                # exp latency instead of stalling the PE queue.
                pt_tiles = {}
                pending_tr = []

                def emit_tr():
                    s_, qtr_, psb_ = pending_tr.pop(0)
                    ps_tr = psp.tile([128, 1024], bf16, tag="tr", bufs=1, name="tr")
                    for j in range(8):
                        nc.tensor.transpose(
                            ps_tr[:, 128 * j:128 * (j + 1)],
                            psb_[:, 128 * j:128 * (j + 1)], ident[:])
                    pt_sb = pb.tile([128, 1024], bf16, tag="pt_sb", bufs=10, name="pt_sb")
                    nc.vector.tensor_copy(pt_sb[:], ps_tr[:])
                    pt_tiles[(s_, qtr_)] = pt_sb

                for qtr in range(4):
                    with nc.named_scope(f"sc{qtr}"):
                        kq = kq_tiles[qtr]
                        for s in range(4):
                            if qtr >= PADQ[s]:
                                continue
                            last_q = (qtr == PADQ[s] - 1)
                            psrc = []  # exp sources per half
                            pstiles = []
                            for pn in range(2):
                                ps = psp.tile([128, 512], f32, tag="pp", name="pp", bufs=3)
                                pstiles.append(ps)
                                for e in range(8):
                                    nc.tensor.matmul(
                                        ps[:],
                                        qt_all[:, 512 * e + 128 * s:512 * e + 128 * (s + 1)],
                                        kq[:, 4096 * pn + 512 * e:4096 * pn + 512 * (e + 1)],
                                        start=(e == 0), stop=(e == 7),
                                    )
                                if last_q:
                                    shift = pb.tile([128, 1], f32, tag="shift", bufs=2, name="shift")
                                    nc.vector.tensor_scalar_add(
                                        shift[:], pos_sb[:, s:s + 1],
                                        float(-(qtr * 1024 + pn * 512)),
                                    )
                                    madd = pb.tile([128, 512], f32, tag="madd", bufs=2, name="madd")
                                    nc.vector.tensor_scalar(
                                        out=madd[:], in0=iota_f[:], scalar1=shift[:, 0:1],
                                        scalar2=NEG_BIG, op0=OP.is_gt, op1=OP.mult,
                                    )
                                    ssb = pb.tile([128, 512], f32, tag="ssb", bufs=2, name="ssb")
                                    nc.vector.tensor_tensor(
                                        out=ssb[:], in0=ps[:], in1=madd[:], op=OP.add)
                                    psrc.append(ssb)
                                else:
                                    psrc.append(ps)
                                if qtr == 0:
                                    # max over MASKED scores (l would underflow to 0
                                    # for short-prefix rows otherwise)
                                    nc.vector.reduce_max(
                                        negm2[s][:, pn:pn + 1], psrc[pn][:], axis=AX.X,
                                        negate=True)
                            if qtr == 0:
                                nc.vector.tensor_tensor(
                                    out=negm[s][:], in0=negm2[s][:, 0:1],
                                    in1=negm2[s][:, 1:2], op=OP.min)
                                nc.vector.tensor_scalar_add(negm[s][:], negm[s][:], -DELTA)
                            psb = pb.tile([128, 1024], bf16, tag="psb", bufs=3, name="psb")
                            for pn in range(2):
                                nc.scalar.activation(
                                    psb[:, 512 * pn:512 * (pn + 1)], psrc[pn][:],
                                    AT.Exp, bias=negm[s][:, 0:1], scale=1.0,
                                    accum_out=lpart[s][:, 2 * qtr + pn:2 * qtr + pn + 1],
                                )
                            if last_q:
                                # 1/l ready well before fin needs it
                                lsum = pb.tile([128, 1], f32, tag="lsum", bufs=2, name="lsum")
                                nc.vector.reduce_sum(
                                    lsum[:], lpart[s][:, 0:2 * PADQ[s]], axis=AX.X)
                                nc.vector.reciprocal(rl_p[s][:], lsum[:])
                            pending_tr.append((s, qtr, psb))
                            if len(pending_tr) > 1:
                                emit_tr()
                while pending_tr:
                    emit_tr()

                # ---- Pass 2: att @ v per quarter, then per-slot finalize ----
                # fin(s) chains PE work behind vector/scalar latency; defer each
                # fin until after the NEXT quarter's av matmuls are issued so
                # those matmuls hide the latency.
                def do_fin(s):
                    with nc.named_scope(f"fin{s}"):
                        attn_b = pb.tile([128, D], bf16, tag="attn_b", bufs=2, name="attn_b")
                        nc.scalar.activation(
                            attn_b[:], A_sb[s][:], AT.Copy, bias=0.0,
                            scale=rl_p[s][:, 0:1])
                        nc.gpsimd.dma_start(
                            out=attn_d[128 * s:128 * (s + 1), :], in_=attn_b[:])
                        ps_t2 = psp.tile([128, 1024], bf16, tag="tr", bufs=1, name="tr")
                        for ec in range(8):
                            nc.tensor.transpose(
                                ps_t2[:, 128 * ec:128 * (ec + 1)],
                                attn_b[:, 128 * ec:128 * (ec + 1)],
                                ident[:])
                        at_row = pb.tile([128, 1024], bf16, tag="at_sb", bufs=2, name="at_sb")
                        nc.vector.tensor_copy(at_row[:], ps_t2[:])
                        out_sb = pb.tile([128, D], bf16, tag="out_sb", bufs=2, name="out_sb")
                        for h in range(2):
                            ps_o = psp.tile([128, 512], f32, tag="pp", name="pp", bufs=3)
                            for ec in range(8):
                                nc.tensor.matmul(
                                    ps_o[:],
                                    at_row[:, 128 * ec:128 * (ec + 1)],
                                    wproj[:, 1024 * ec + 512 * h:1024 * ec + 512 * (h + 1)],
                                    start=(ec == 0), stop=(ec == 7),
                                )
                            nc.vector.tensor_tensor(
                                out=out_sb[:, 512 * h:512 * (h + 1)], in0=ps_o[:],
                                in1=xqr[:, 1024 * s + 512 * h:1024 * s + 512 * (h + 1)],
                                op=OP.add)
                        nc.gpsimd.dma_start(
                            out=out_d[128 * s:128 * (s + 1), :], in_=out_sb[:])

                pending_fin = []
                for qtr in range(4):
                    with nc.named_scope(f"av{qtr}"):
                        vq = vq_tiles[qtr]
                        for s in range(4):
                            if qtr >= PADQ[s]:
                                continue
                            pt_sb = pt_tiles[(s, qtr)]
                            ps_av = psp.tile([128, 1024], f32, tag="av", name="av", bufs=2)
                            for h in range(2):
                                for j in range(8):
                                    nc.tensor.matmul(
                                        ps_av[:, 512 * h:512 * (h + 1)],
                                        pt_sb[:, 128 * j:128 * (j + 1)],
                                        vq[:, 1024 * j + 512 * h:1024 * j + 512 * (h + 1)],
                                        start=(j == 0), stop=(j == 7),
                                    )
                            if qtr == 0:
                                nc.vector.tensor_copy(A_sb[s][:], ps_av[:])
                            else:
                                nc.vector.tensor_tensor(
                                    out=A_sb[s][:], in0=A_sb[s][:], in1=ps_av[:], op=OP.add)
                        while pending_fin:
                            do_fin(pending_fin.pop(0))
                        pending_fin.append(qtr)  # slot s==qtr finishes at qtr
                while pending_fin:
                    do_fin(pending_fin.pop(0))

    nc.compile()
    return nc


def _get_compiled():
    global _COMPILED
    if _COMPILED is None:
        _COMPILED = _build()
    return _COMPILED


def kernel(x, attention_mask, Wq, Wkv, Wproj, _trace=False):
    global LAST_EXEC_NS, LAST_RES
    from concourse.bass_utils import run_bass_kernel_spmd

    x = np.asarray(x)
    attention_mask = np.asarray(attention_mask)
    Wq, Wkv, Wproj = np.asarray(Wq), np.asarray(Wkv), np.asarray(Wproj)
    assert x.shape == (T, D) and attention_mask.shape == (T,)
    assert np.array_equal(attention_mask, np.arange(T, dtype=attention_mask.dtype)), \
        "kernel assumes attention_mask == arange(T)"

    x16 = x.astype(np.float16)
    wqT = np.ascontiguousarray(Wq.T).astype(np.float16)
    wkT = np.ascontiguousarray(Wkv[:D].T).astype(np.float16)
    wvT = np.ascontiguousarray(Wkv[D:].T).astype(np.float16)
    wpT = np.ascontiguousarray(Wproj.T).astype(ml_dtypes.bfloat16)

    in_maps = []
    core_rows = []
    for c in range(N_CORES):
        blocks = core_blocks(c)
        rows = np.concatenate([np.arange(128 * b, 128 * (b + 1)) for b in blocks])
        core_rows.append(rows)
        xqT = np.ascontiguousarray(x16[rows].T)    # [D, 512] f16
        xkT = np.ascontiguousarray(x16[512 * c:512 * (c + 1)].T)
        vrows = np.concatenate([np.arange(128 * b, 128 * (b + 1)) for b in kv_blocks(c)])
        xvT = np.ascontiguousarray(x16[vrows].T)
        pos = np.empty((128, 4), np.float32)
        for s, b in enumerate(blocks):
            pos[:, s] = 128 * b + np.arange(128)
        in_maps.append({
            "xqT": xqT, "xkT": xkT, "xvT": xvT,
            "xqres": np.ascontiguousarray(x16[rows]),
            "pos": pos,
            "wqT": wqT, "wkT": wkT, "wvT": wvT, "wpT": wpT,
        })

    nc = _get_compiled()
    res = run_bass_kernel_spmd(nc, in_maps, list(range(N_CORES)), trace=_trace)
    LAST_EXEC_NS = res.exec_time_ns
    LAST_RES = res

    out_full = np.empty((T, D), np.float32)
    x_new = x.astype(np.float32).copy()
    for c in range(N_CORES):
        r = res.results[c]
        out_full[core_rows[c]] = r["out"].astype(np.float32)
        x_new[core_rows[c]] += r["attn"].astype(np.float32)
    return out_full, x_new
